# revision 25
# baseline (speedup 1.0000x reference)
"""Trainium2 Bass kernel for nn_MetaGraphLearner (GNN edge scorer).

Math (reference):
  t  = X @ Wt.T + bt                  [B,N,H]
  hi = t @ W1i.T, hj = t @ W1j.T      [B,N,E]   (W1 = [W1i | W1j])
  ew[b,i,j] = sum_e W2[e]*relu(hi[b,i,e]+hj[b,j,e]+b1[e]) + b2
  out = sigmoid(ew) * adj[None]

Wt is folded into W1 host-side (weight algebra only):
  hj[j,e] = X[j] @ (W1j@Wt)[e], hi likewise; all constant terms fold into one
  per-(k,e) bias vector bvec (shipped as a bf16 hi/lo pair, recombined in f32).

Kernel (per core, 8 cores, each owns 256 of the B*N=2048 rows):
  Partition layout p = 32*k + e  (k in 0..3 row-sublane, e in 0..31).
  hjT4[p, j]  = |W2[e]|*hj_lin[j,e]             [128,1024] bf16 (2 matmuls +
                PSUM->SBUF copies on Act/DVE)
  hb[p, g]    = |W2[e]|*hi_lin[4g+k,e] + bvec   [128,64] f32 (4 matmuls + an
                Identity activation w/ bias on Act)
  R_g[p, j]   = max(hjT4[p,j] + hb[p,g], 0)     64 groups, split 39/10/15
                across DVE (4x bf16 mode, 327ns), Act (relu w/ bias, 1038ns)
                and GpSimd/Pool (853ns) to finish simultaneously
  psum[i', j] += sign(W2[e]) * R_g[p,j]         8 bf16 matmuls per group into
                six PSUM sub-banks (BANKS table) sized 16/16/16/8/4/4 groups
                so output chunks complete at g15/31/47/55/59/63
  mask        : since adj is 0/1, sigmoid(ew)*adj == sigmoid(ew - 1e9*(1-adj));
                M = (adj-1)*1e9 is computed from bf16 adj (DVE 4x, one slice
                on Pool) and seeded into each PSUM sub-bank by ONE
                identity-stationary matmul (start=True) BEFORE the group
                matmuls accumulate (start=False). No mask multiply exists.
  out         = sigmoid(psum + b2)  ->  o_sb [j_loc, 256*js + i'], six
                activation chunks firing as sub-banks complete; 5 output DMAs
                cascade so only a tiny [*, 16-col] piece (issued by Act right
                after the last sigmoid) remains after g63.

Cost-model notes (CoreSim): matmuls are charged out-free-size * 0.42ns only
(stationary load unmodeled) so PE is nearly free; DMA HWDGE descriptor-gen
(625ns) and the transfer pool are globally serialized, hence few, ordered
DMAs with madj routed via Pool SWDGE; every DMA completion costs +900ns sem
propagation. Walrus constraints found by probing: GpSimd must not touch PSUM;
activation inputs must collapse to <=2D APs.

Raw Bass (explicit engine blocks + semaphores): this walrus build rejects >1
attached sync wait per instruction, so cross-engine waits are standalone
wait_ge instructions.
"""

import sys

if "/opt/trn_rl_repo" not in sys.path:
    sys.path.insert(0, "/opt/trn_rl_repo")

import numpy as np
from contextlib import ExitStack

B, N, H, E = 2, 1024, 128, 32
NCORES = 8
ROWS_PER_CORE = (B * N) // NCORES  # 256
NG = 64                   # groups of 4 rows per core
RBUF = 12                 # R-tile ring slots
NSCA = 10                 # relu tiles on ScalarE (Act)
NPOOL = 15                # relu tiles on GpSimd (Pool)

# bf16 input "xtb" free-axis layout: [ X.T (N) | X2.T (256) ]
XTB_W = N + ROWS_PER_CORE
# bf16 input "wb" layout: sgn(4) | wc(128) | wci(32) | bv_hi | bv_lo | b2 | ident(128)
SGN_O, WC_O, WCI_O, BV_O, ID_O = 0, 4, 132, 164, 167
WB_W = ID_O + 128  # 295

N_PREP_MM = 6  # 2 hj + 4 hb (these inc sem_pe; M-matmuls do not)

# --- relu group -> engine assignment -----------------------------------------
# deterministic largest-deficit interleave with exact counts
_counts = {"v": NG - NSCA - NPOOL, "a": NSCA, "p": NPOOL}
_assign = []
_used = {"v": 0, "a": 0, "p": 0}
for g in range(NG):
    best = max(("v", "a", "p"), key=lambda e: (_counts[e] * (g + 1) / NG - _used[e], e))
    _assign.append(best)
    _used[best] += 1
# force the last 3 groups onto DVE (fastest) so the tail starts ASAP
for g in (60, 61, 62, 63):
    if _assign[g] != "v":
        for g2 in range(60, -1, -1):
            if _assign[g2] == "v":
                _assign[g2], _assign[g] = _assign[g], "v"
                break
SCA_GG = [g for g in range(NG) if _assign[g] == "a"]
POOL_GG = [g for g in range(NG) if _assign[g] == "p"]
VEC_GG = [g for g in range(NG) if _assign[g] == "v"]

# PSUM sub-banks (each its own physical 2KB bank so the M-seed + walrus's
# 2D-activation-input constraint hold): (first group, #groups, bank cols, madj col base)
# within-bank col = (#groups/2)*js*?? -> col = NJ*js + 4*glh' + k with NJ = 4*ngroups/8*4
BANKS = [
    (0, 16, 512, 0),       # slab0 gh0: groups 0-15
    (16, 16, 512, 512),    # slab0 gh1
    (32, 16, 512, 1024),   # slab1 gh0
    (48, 8, 256, 1536),    # slab1 gh1 glh 0-7
    (56, 4, 128, 1792),    # glh 8-11
    (60, 4, 128, 1920),    # glh 12-15
]
# sigmoid chunks, one per bank: (g_end, bank idx, o_sb i-col base, i width)
SIG_CHUNKS = [
    (15, 0, 0, 64),
    (31, 1, 64, 64),
    (47, 2, 128, 64),
    (55, 3, 192, 32),
    (59, 4, 224, 16),
    (63, 5, 240, 16),
]
# out DMA pieces: (#sig chunks required, o_sb i-col base, width)
OUT_PIECES = [(2, 0, 128), (3, 128, 64), (4, 192, 32), (5, 224, 16), (6, 240, 16)]

_CACHE = {}


def _build_program():
    import concourse.bass as bass
    import concourse.mybir as mybir

    f32 = mybir.dt.float32
    bf16 = mybir.dt.bfloat16
    AF = mybir.ActivationFunctionType
    ALU = mybir.AluOpType

    nc = bass.Bass()
    xtb = nc.declare_dram_parameter("xtb", [128, XTB_W], bf16, isOutput=False)
    wb = nc.declare_dram_parameter("wb", [128, WB_W], bf16, isOutput=False)
    madj = nc.declare_dram_parameter("madj", [128, 2048], bf16, isOutput=False)
    out_d = nc.declare_dram_parameter("out", [N, ROWS_PER_CORE], f32, isOutput=True)

    # per-engine producer ordinals for the relu ring
    vcnt, scnt, pcnt = {}, {}, {}
    v = s = p = 0
    for g in range(NG):
        if _assign[g] == "a":
            s += 1
            scnt[g] = s
        elif _assign[g] == "p":
            p += 1
            pcnt[g] = p
        else:
            v += 1
            vcnt[g] = v

    with ExitStack() as ctx:
        EN = ctx.enter_context
        xtb_sb = EN(nc.sbuf_tensor("xtb_sb", [128, XTB_W], bf16))
        wb_sb = EN(nc.sbuf_tensor("wb_sb", [128, WB_W], bf16))
        cvt_sb = EN(nc.sbuf_tensor("cvt_sb", [128, 3], f32))
        warm_sb = EN(nc.sbuf_tensor("warm_sb", [128, 1], f32))
        wmm_sb = EN(nc.sbuf_tensor("wmm_sb", [128, 4], bf16))
        hj_sb = EN(nc.sbuf_tensor("hj_sb", [128, N], bf16))
        hb_sb = EN(nc.sbuf_tensor("hb_sb", [128, NG], f32))
        madj_sb = EN(nc.sbuf_tensor("madj_sb", [128, 2048], bf16))
        m_sb = EN(nc.sbuf_tensor("m_sb", [128, 2048], bf16))
        r_sb = [EN(nc.sbuf_tensor(f"r{i}", [128, N], bf16)) for i in range(RBUF)]
        o_sb = EN(nc.sbuf_tensor("o_sb", [128, 2048], f32))

        acc = [EN(nc.psum_tensor(f"acc{i}", [128, 512], f32)) for i in range(6)]

        sem_xta = EN(nc.semaphore("sxta"))
        sem_xtb = EN(nc.semaphore("sxtb"))
        sem_xt2 = EN(nc.semaphore("sxt2"))
        sem_wb = EN(nc.semaphore("swb"))
        sem_ma0 = EN(nc.semaphore("sma0"))
        sem_ma1 = EN(nc.semaphore("sma1"))
        sem_cvt = EN(nc.semaphore("scvt"))
        sem_hb = EN(nc.semaphore("shb"))
        sem_mop = EN(nc.semaphore("smop"))
        sem_mop2 = EN(nc.semaphore("smop2"))
        sem_pe = EN(nc.semaphore("spe"))
        sem_h0 = EN(nc.semaphore("sh0"))
        sem_vec = EN(nc.semaphore("svec"))
        sem_sig = EN(nc.semaphore("ssig"))
        sem_vR = EN(nc.semaphore("svr"))
        sem_sR = EN(nc.semaphore("ssr"))
        sem_pR = EN(nc.semaphore("spr"))
        sem_warm = EN(nc.semaphore("swarm"))
        sem_pms = EN(nc.semaphore("spms"))
        sem_out = EN(nc.semaphore("sout"))

        xt_a = xtb_sb[:, 0:N]
        sgn_a = wb_sb[:, SGN_O:SGN_O + 4]
        wc_a = wb_sb[:, WC_O:WC_O + 128]
        wci_a = wb_sb[:, WCI_O:WCI_O + E]
        id_a = wb_sb[:, ID_O:ID_O + 128]
        bvh_a = cvt_sb[:, 0:1]
        bvl_a = cvt_sb[:, 1:2]
        b2_a = cvt_sb[:, 2:3]

        o_v = o_sb[:].rearrange("p (js i) -> p js i", js=8)
        out_v = out_d[:].rearrange("(js p) i -> p js i", js=8)

        block = EN(nc.Block())

        def r_war_wait(eng, g):
            # overwrite slot g%RBUF: previous tenant g-RBUF must be consumed
            if g >= RBUF:
                eng.wait_ge(sem_pe, N_PREP_MM + 8 * (g - RBUF + 1))

        def first_waits(eng, need):
            # hj full + hb ready before any relu
            for sem_, n in need:
                eng.wait_ge(sem_, n)

        @block.sync
        def _(sp):
            sp.dma_start(xtb_sb[:, 0:512], xtb[:, 0:512]).then_inc(sem_xta, 16)
            sp.dma_start(xtb_sb[:, 512:N], xtb[:, 512:N]).then_inc(sem_xtb, 16)
            sp.dma_start(xtb_sb[:, N:XTB_W], xtb[:, N:XTB_W]).then_inc(sem_xt2, 16)
            sp.dma_start(madj_sb[:, 1024:2048], madj[:, 1024:2048]).then_inc(sem_ma1, 16)
            for nsig, ibase, w in OUT_PIECES[:-1]:
                sp.wait_ge(sem_sig, nsig)
                sp.dma_start(
                    out_v[:, :, ibase:ibase + w], o_v[:, :, ibase:ibase + w],
                ).then_inc(sem_out, 16)

        @block.gpsimd
        def _(gp):
            gp.memset(wmm_sb[:], 0.0).then_inc(sem_pms, 1)
            gp.wait_ge(sem_wb, 16)
            gp.dma_start(madj_sb[:, 0:1024], madj[:, 0:1024]).then_inc(sem_ma0, 16)
            nc.gpsimd.tensor_tensor(
                cvt_sb[:, 0:1], wb_sb[:, BV_O:BV_O + 1], wb_sb[:, BV_O + 1:BV_O + 2],
                ALU.add,
            )
            nc.gpsimd.tensor_copy(cvt_sb[:, 2:3], wb_sb[:, BV_O + 2:BV_O + 3]).then_inc(sem_cvt, 1)
            first = True
            for i_, g in enumerate(POOL_GG):
                if first:
                    first_waits(gp, [(sem_h0, 1), (sem_vec, 1), (sem_hb, 1)])
                    first = False
                r_war_wait(gp, g)
                nc.gpsimd.tensor_scalar(
                    r_sb[g % RBUF][:], hj_sb[:], hb_sb[:, g:g + 1], 0.0,
                    ALU.add, ALU.max,
                ).then_inc(sem_pR, 1)
                if i_ == 2:
                    gp.wait_ge(sem_ma1, 16)
                    nc.gpsimd.tensor_scalar(
                        m_sb[:, 1536:2048], madj_sb[:, 1536:2048], -1.0, 1e9,
                        ALU.add, ALU.mult,
                    ).then_inc(sem_mop2, 1)

        @block.tensor
        def _(pe):
            pe.wait_ge(sem_pms, 1)
            nc.tensor.matmul(acc[0][0:2, 0:2], wmm_sb[:, 0:2], wmm_sb[:, 2:4])
            pe.wait_ge(sem_xta, 16)
            pe.wait_ge(sem_wb, 16)
            nc.tensor.matmul(acc[0][:], wc_a, xt_a[:, 0:512]).then_inc(sem_pe, 1)
            pe.wait_ge(sem_xtb, 16)
            nc.tensor.matmul(acc[1][:], wc_a, xt_a[:, 512:1024]).then_inc(sem_pe, 1)
            pe.wait_ge(sem_xt2, 16)
            xt2_v = xtb_sb[:, N:XTB_W].rearrange("p (g k) -> p g k", k=4)
            for k in range(4):
                nc.tensor.matmul(
                    acc[2][32 * k:32 * (k + 1), 0:NG], wci_a, xt2_v[:, :, k],
                    tile_position=(0, 32 * k),
                ).then_inc(sem_pe, 1)
            # main: R stationary vs sign strip; bank cols NJ*js + 4*glh' + k.
            # Each sub-bank is seeded with the mask bias M (start=True) first.
            for rb, (g0, ngr, bw, mcb) in enumerate(BANKS):
                nj = bw // 8
                if rb == 0:
                    pe.wait_ge(sem_mop, 1)
                    pe.wait_ge(sem_h0, 1)   # copy h0 read of acc0 done
                elif rb == 1:
                    pe.wait_ge(sem_vec, 1)  # copy h1 read of acc1 done
                elif rb == 2:
                    pe.wait_ge(sem_mop, 2)
                    pe.wait_ge(sem_mop2, 1)
                    pe.wait_ge(sem_hb, 1)   # hb-add read of acc2 done
                nc.tensor.matmul(
                    acc[rb][:, 0:bw], id_a, m_sb[:, mcb:mcb + bw],
                    start=True, stop=False, skip_group_check=True,
                )
                for g in range(g0, g0 + ngr):
                    glh = g - g0
                    if _assign[g] == "a":
                        pe.wait_ge(sem_sR, scnt[g])
                    elif _assign[g] == "p":
                        pe.wait_ge(sem_pR, pcnt[g])
                    else:
                        pe.wait_ge(sem_vR, vcnt[g])
                    r = r_sb[g % RBUF]
                    for js in range(8):
                        col = nj * js + 4 * glh
                        last = glh == ngr - 1 and js == 7
                        nc.tensor.matmul(
                            acc[rb][:, col:col + 4],
                            r[:, 128 * js:128 * (js + 1)], sgn_a,
                            start=False, stop=last, skip_group_check=True,
                        ).then_inc(sem_pe, 1)

        @block.scalar
        def _(sc):
            sc.dma_start(wb_sb[:], wb[:]).then_inc(sem_wb, 16)
            nc.scalar.memzero(warm_sb[:]).then_inc(sem_warm, 1)
            sc.wait_ge(sem_warm, 1)
            nc.scalar.activation(warm_sb[:], warm_sb[:], AF.Sigmoid)
            sc.wait_ge(sem_pe, 1)
            nc.scalar.activation(
                hj_sb[:, 0:512], acc[0][:], AF.Copy,
            ).then_inc(sem_h0, 1)
            sc.wait_ge(sem_cvt, 1)
            sc.wait_ge(sem_pe, N_PREP_MM)
            nc.scalar.activation(
                hb_sb[:], acc[2][:, 0:NG], AF.Identity, bias=bvh_a, scale=1.0,
            ).then_inc(sem_hb, 1)

            sig_i = [0]

            def sigs(now_g):
                while sig_i[0] < len(SIG_CHUNKS):
                    g_end, rb, ib, w = SIG_CHUNKS[sig_i[0]]
                    lead = 5 if sig_i[0] < 4 else 2
                    if now_g < g_end + lead and now_g < NG:
                        return
                    if sig_i[0] == 0:
                        sc.wait_ge(sem_cvt, 1)
                    sc.wait_ge(sem_pe, N_PREP_MM + 8 * (g_end + 1))
                    bw = BANKS[rb][2]
                    nc.scalar.activation(
                        o_v[:, :, ib:ib + w], acc[rb][:, 0:bw],
                        AF.Sigmoid, bias=b2_a, scale=1.0,
                    ).then_inc(sem_sig, 1)
                    sig_i[0] += 1

            first = True
            for g in SCA_GG:
                if first:
                    first_waits(sc, [(sem_h0, 1), (sem_vec, 1), (sem_hb, 1)])
                    first = False
                r_war_wait(sc, g)
                nc.scalar.activation(
                    r_sb[g % RBUF][:], hj_sb[:], AF.Relu,
                    bias=hb_sb[:, g:g + 1], scale=1.0,
                ).then_inc(sem_sR, 1)
                sigs(g)
            sigs(NG)
            nsig, ibase, w = OUT_PIECES[-1]
            sc.wait_ge(sem_sig, nsig)
            sc.dma_start(
                out_v[:, :, ibase:ibase + w], o_v[:, :, ibase:ibase + w],
            ).then_inc(sem_out, 16)

        @block.vector
        def _(ve):
            ve.wait_ge(sem_pe, 2)
            nc.vector.tensor_copy(
                hj_sb[:, 512:1024], acc[1][:],
            ).then_inc(sem_vec, 1)
            mop_i = [0]

            def mops(now_g):
                # M = (adj - 1) * 1e9 in bf16, halves gated on the madj DMAs
                if mop_i[0] == 0 and now_g >= 5:
                    ve.wait_ge(sem_ma0, 16)
                    nc.vector.tensor_scalar(
                        m_sb[:, 0:1024], madj_sb[:, 0:1024], -1.0, 1e9,
                        ALU.add, ALU.mult,
                    ).then_inc(sem_mop, 1)
                    mop_i[0] = 1
                if mop_i[0] == 1 and now_g >= 20:
                    ve.wait_ge(sem_ma1, 16)
                    nc.vector.tensor_scalar(
                        m_sb[:, 1024:1536], madj_sb[:, 1024:1536], -1.0, 1e9,
                        ALU.add, ALU.mult,
                    ).then_inc(sem_mop, 1)
                    mop_i[0] = 2

            first = True
            for g in VEC_GG:
                if first:
                    first_waits(ve, [(sem_h0, 1), (sem_vec, 1), (sem_hb, 1)])
                    first = False
                r_war_wait(ve, g)
                nc.vector.tensor_scalar(
                    r_sb[g % RBUF][:], hj_sb[:], hb_sb[:, g:g + 1], 0.0,
                    ALU.add, ALU.max,
                ).then_inc(sem_vR, 1)
                mops(g)
            mops(NG)

    return nc


def _host_prep(node_features, adjacency_matrix, Wt, bt, W1, b1, W2, b2):
    """Build per-core input maps (numpy only: resharding + weight algebra)."""
    import ml_dtypes

    f = np.float32
    bf = ml_dtypes.bfloat16
    W2v = np.asarray(W2, f)[0]                 # [E]
    aW2 = np.abs(W2v)
    sW2 = np.sign(W2v).astype(f)
    W1 = np.asarray(W1, f)
    W1i, W1j = W1[:, :H], W1[:, H:]            # [E, H]
    Wt = np.asarray(Wt, f)                     # [o, h]
    bt = np.asarray(bt, f)
    b1 = np.asarray(b1, f)
    b2c = float(np.asarray(b2, f).reshape(-1)[0])

    WtW1j = W1j @ Wt                           # [E, H]
    WtW1i = W1i @ Wt
    allb = b1 + W1j @ bt + W1i @ bt            # [E] constant part of relu arg

    bvec = np.tile(aW2 * allb, 4).astype(f)    # [128]
    bv_hi = bvec.astype(bf)
    bv_lo = (bvec - bv_hi.astype(f)).astype(bf)

    wb = np.zeros((128, WB_W), bf)
    for k in range(4):
        wb[32 * k:32 * (k + 1), SGN_O + k] = sW2.astype(bf)
    wb[:, WC_O:WC_O + 128] = np.tile((aW2[:, None] * WtW1j).T, (1, 4)).astype(bf)
    wb[:, WCI_O:WCI_O + E] = (aW2[:, None] * WtW1i).T.astype(bf)
    wb[:, BV_O] = bv_hi
    wb[:, BV_O + 1] = bv_lo
    wb[:, BV_O + 2] = bf(b2c)
    wb[:, ID_O:ID_O + 128] = np.eye(128, dtype=bf)

    in_maps = []
    adjacency_matrix = np.asarray(adjacency_matrix, f)
    node_features = np.asarray(node_features, f)
    for c in range(NCORES):
        b, i0 = c // (NCORES // B), ROWS_PER_CORE * (c % (NCORES // B))
        x = node_features[b]
        xtb = np.empty((128, XTB_W), bf)
        xtb[:, 0:N] = x.T.astype(bf)
        xtb[:, N:] = x[i0:i0 + ROWS_PER_CORE].T.astype(bf)
        # madj[j_loc, mcb + NJ*js + 4*glh' + k] = adj[i0 + i_loc, 128*js + j_loc]
        # with i_loc = 4*(g0 + glh') + k for each sub-bank (g0, ngr, bw, mcb)
        adjT = adjacency_matrix[i0:i0 + ROWS_PER_CORE].T.astype(bf)  # [j, i_loc]
        A = adjT.reshape(8, 128, 64, 4)  # js, j_loc, g, k
        madj = np.empty((128, 2048), bf)
        for g0, ngr, bw, mcb in BANKS:
            blk = A[:, :, g0:g0 + ngr, :]          # js, j_loc, glh', k
            blk = blk.transpose(1, 0, 2, 3).reshape(128, bw)  # j_loc, (js, glh', k)
            madj[:, mcb:mcb + bw] = blk
        in_maps.append(dict(xtb=xtb, wb=wb, madj=madj))
    return in_maps


def run(inputs, trace=False):
    from concourse.bass_utils import run_bass_kernel_spmd

    if "prog" not in _CACHE:
        _CACHE["prog"] = _build_program()
    nc = _CACHE["prog"]
    in_maps = _host_prep(**inputs)
    res = run_bass_kernel_spmd(nc, in_maps, list(range(NCORES)), trace=trace)
    out = np.empty((B, N, N), np.float32)
    for c in range(NCORES):
        b, i0 = c // (NCORES // B), ROWS_PER_CORE * (c % (NCORES // B))
        out[b, i0:i0 + ROWS_PER_CORE] = res.results[c]["out"].T
    return out, res


def kernel(**inputs):
    out, _ = run(inputs, trace=False)
    return out


# revision 26
# speedup vs baseline: 1.0024x; 1.0024x over previous
"""Trainium2 Bass kernel for nn_MetaGraphLearner (GNN edge scorer).

Math (reference):
  t  = X @ Wt.T + bt                  [B,N,H]
  hi = t @ W1i.T, hj = t @ W1j.T      [B,N,E]   (W1 = [W1i | W1j])
  ew[b,i,j] = sum_e W2[e]*relu(hi[b,i,e]+hj[b,j,e]+b1[e]) + b2
  out = sigmoid(ew) * adj[None]

Wt is folded into W1 host-side (weight algebra only):
  hj[j,e] = X[j] @ (W1j@Wt)[e], hi likewise; all constant terms fold into one
  per-(k,e) bias vector bvec (shipped as a bf16 hi/lo pair, recombined in f32).

Kernel (per core, 8 cores, each owns 256 of the B*N=2048 rows):
  Partition layout p = 32*k + e  (k in 0..3 row-sublane, e in 0..31).
  hjT4[p, j]  = |W2[e]|*hj_lin[j,e]             [128,1024] bf16 (2 matmuls +
                PSUM->SBUF copies on Act/DVE)
  hb[p, g]    = |W2[e]|*hi_lin[4g+k,e] + bvec   [128,64] f32 (4 matmuls + an
                Identity activation w/ bias on Act)
  R_g[p, j]   = max(hjT4[p,j] + hb[p,g], 0)     64 groups, split 39/10/15
                across DVE (4x bf16 mode, 327ns), Act (relu w/ bias, 1038ns)
                and GpSimd/Pool (853ns) to finish simultaneously
  psum[i', j] += sign(W2[e]) * R_g[p,j]         8 bf16 matmuls per group into
                six PSUM sub-banks (BANKS table) sized 16/16/16/8/4/4 groups
                so output chunks complete at g15/31/47/55/59/63
  mask        : since adj is 0/1, sigmoid(ew)*adj == sigmoid(ew - 1e9*(1-adj));
                M = (adj-1)*1e9 is computed from bf16 adj (DVE 4x, one slice
                on Pool) and seeded into each PSUM sub-bank by ONE
                identity-stationary matmul (start=True) BEFORE the group
                matmuls accumulate (start=False). No mask multiply exists.
  out         = sigmoid(psum + b2)  ->  o_sb [j_loc, 256*js + i'], six
                activation chunks firing as sub-banks complete; 5 output DMAs
                cascade so only a tiny [*, 16-col] piece (issued by Act right
                after the last sigmoid) remains after g63.

Cost-model notes (CoreSim): matmuls are charged out-free-size * 0.42ns only
(stationary load unmodeled) so PE is nearly free; DMA HWDGE descriptor-gen
(625ns) and the transfer pool are globally serialized, hence few, ordered
DMAs with madj routed via Pool SWDGE; every DMA completion costs +900ns sem
propagation. Walrus constraints found by probing: GpSimd must not touch PSUM;
activation inputs must collapse to <=2D APs.

Raw Bass (explicit engine blocks + semaphores): this walrus build rejects >1
attached sync wait per instruction, so cross-engine waits are standalone
wait_ge instructions.
"""

import sys

if "/opt/trn_rl_repo" not in sys.path:
    sys.path.insert(0, "/opt/trn_rl_repo")

import numpy as np
from contextlib import ExitStack

B, N, H, E = 2, 1024, 128, 32
NCORES = 8
ROWS_PER_CORE = (B * N) // NCORES  # 256
NG = 64                   # groups of 4 rows per core
RBUF = 12                 # R-tile ring slots
NSCA = 10                 # relu tiles on ScalarE (Act)
NPOOL = 15                # relu tiles on GpSimd (Pool)

# bf16 input "xtb" free-axis layout: [ X.T (N) | X2.T (256) ]
XTB_W = N + ROWS_PER_CORE
# bf16 input "wb" layout: sgn(4) | wc(128) | wci(32) | bv_hi | bv_lo | b2 | ident(128)
SGN_O, WC_O, WCI_O, BV_O, ID_O = 0, 4, 132, 164, 167
WB_W = ID_O + 128  # 295

N_PREP_MM = 6  # 2 hj + 4 hb (these inc sem_pe; M-matmuls do not)

# --- relu group -> engine assignment -----------------------------------------
# deterministic largest-deficit interleave with exact counts
_counts = {"v": NG - NSCA - NPOOL, "a": NSCA, "p": NPOOL}
_assign = []
_used = {"v": 0, "a": 0, "p": 0}
for g in range(NG):
    best = max(("v", "a", "p"), key=lambda e: (_counts[e] * (g + 1) / NG - _used[e], e))
    _assign.append(best)
    _used[best] += 1
# force the last 3 groups onto DVE (fastest) so the tail starts ASAP
for g in (60, 61, 62, 63):
    if _assign[g] != "v":
        for g2 in range(60, -1, -1):
            if _assign[g2] == "v":
                _assign[g2], _assign[g] = _assign[g], "v"
                break
SCA_GG = [g for g in range(NG) if _assign[g] == "a"]
POOL_GG = [g for g in range(NG) if _assign[g] == "p"]
VEC_GG = [g for g in range(NG) if _assign[g] == "v"]

# PSUM sub-banks (each its own physical 2KB bank so the M-seed + walrus's
# 2D-activation-input constraint hold): (first group, #groups, bank cols, madj col base)
# within-bank col = (#groups/2)*js*?? -> col = NJ*js + 4*glh' + k with NJ = 4*ngroups/8*4
BANKS = [
    (0, 16, 512, 0),       # slab0 gh0: groups 0-15
    (16, 16, 512, 512),    # slab0 gh1
    (32, 16, 512, 1024),   # slab1 gh0
    (48, 8, 256, 1536),    # slab1 gh1 glh 0-7
    (56, 4, 128, 1792),    # glh 8-11
    (60, 4, 128, 1920),    # glh 12-15
]
# sigmoid chunks, one per bank: (g_end, bank idx, o_sb i-col base, i width)
SIG_CHUNKS = [
    (15, 0, 0, 64),
    (31, 1, 64, 64),
    (47, 2, 128, 64),
    (55, 3, 192, 32),
    (59, 4, 224, 16),
    (63, 5, 240, 16),
]
# out DMA pieces: (#sig chunks required, o_sb i-col base, width)
OUT_PIECES = [(2, 0, 128), (3, 128, 64), (4, 192, 32), (5, 224, 16), (6, 240, 16)]

_CACHE = {}


def _build_program():
    import concourse.bass as bass
    import concourse.mybir as mybir

    f32 = mybir.dt.float32
    bf16 = mybir.dt.bfloat16
    AF = mybir.ActivationFunctionType
    ALU = mybir.AluOpType

    nc = bass.Bass()
    xtb = nc.declare_dram_parameter("xtb", [128, XTB_W], bf16, isOutput=False)
    wb = nc.declare_dram_parameter("wb", [128, WB_W], bf16, isOutput=False)
    madj = nc.declare_dram_parameter("madj", [128, 2048], bf16, isOutput=False)
    out_d = nc.declare_dram_parameter("out", [N, ROWS_PER_CORE], f32, isOutput=True)

    # per-engine producer ordinals for the relu ring. Pool's first group is
    # produced as two halves (two sem_pR increments).
    vcnt, scnt, pcnt = {}, {}, {}
    v = s = p = 0
    pool_first = POOL_GG[0]
    for g in range(NG):
        if _assign[g] == "a":
            s += 1
            scnt[g] = s
        elif _assign[g] == "p":
            p += 2 if g == pool_first else 1
            pcnt[g] = p
        else:
            v += 1
            vcnt[g] = v

    with ExitStack() as ctx:
        EN = ctx.enter_context
        xtb_sb = EN(nc.sbuf_tensor("xtb_sb", [128, XTB_W], bf16))
        wb_sb = EN(nc.sbuf_tensor("wb_sb", [128, WB_W], bf16))
        cvt_sb = EN(nc.sbuf_tensor("cvt_sb", [128, 3], f32))
        warm_sb = EN(nc.sbuf_tensor("warm_sb", [128, 1], f32))
        wmm_sb = EN(nc.sbuf_tensor("wmm_sb", [128, 4], bf16))
        hj_sb = EN(nc.sbuf_tensor("hj_sb", [128, N], bf16))
        hb_sb = EN(nc.sbuf_tensor("hb_sb", [128, NG], f32))
        madj_sb = EN(nc.sbuf_tensor("madj_sb", [128, 2048], bf16))
        m_sb = EN(nc.sbuf_tensor("m_sb", [128, 2048], bf16))
        r_sb = [EN(nc.sbuf_tensor(f"r{i}", [128, N], bf16)) for i in range(RBUF)]
        o_sb = EN(nc.sbuf_tensor("o_sb", [128, 2048], f32))

        acc = [EN(nc.psum_tensor(f"acc{i}", [128, 512], f32)) for i in range(6)]

        sem_xta = EN(nc.semaphore("sxta"))
        sem_xtb = EN(nc.semaphore("sxtb"))
        sem_xt2 = EN(nc.semaphore("sxt2"))
        sem_wb = EN(nc.semaphore("swb"))
        sem_ma0 = EN(nc.semaphore("sma0"))
        sem_ma1 = EN(nc.semaphore("sma1"))
        sem_cvt = EN(nc.semaphore("scvt"))
        sem_hb = EN(nc.semaphore("shb"))
        sem_mop = EN(nc.semaphore("smop"))
        sem_mop2 = EN(nc.semaphore("smop2"))
        sem_pe = EN(nc.semaphore("spe"))
        sem_h0 = EN(nc.semaphore("sh0"))
        sem_vec = EN(nc.semaphore("svec"))
        sem_sig = EN(nc.semaphore("ssig"))
        sem_vR = EN(nc.semaphore("svr"))
        sem_sR = EN(nc.semaphore("ssr"))
        sem_pR = EN(nc.semaphore("spr"))
        sem_warm = EN(nc.semaphore("swarm"))
        sem_pms = EN(nc.semaphore("spms"))
        sem_out = EN(nc.semaphore("sout"))

        xt_a = xtb_sb[:, 0:N]
        sgn_a = wb_sb[:, SGN_O:SGN_O + 4]
        wc_a = wb_sb[:, WC_O:WC_O + 128]
        wci_a = wb_sb[:, WCI_O:WCI_O + E]
        id_a = wb_sb[:, ID_O:ID_O + 128]
        bvh_a = cvt_sb[:, 0:1]
        bvl_a = cvt_sb[:, 1:2]
        b2_a = cvt_sb[:, 2:3]

        o_v = o_sb[:].rearrange("p (js i) -> p js i", js=8)
        out_v = out_d[:].rearrange("(js p) i -> p js i", js=8)

        block = EN(nc.Block())

        def r_war_wait(eng, g):
            # overwrite slot g%RBUF: previous tenant g-RBUF must be consumed
            if g >= RBUF:
                eng.wait_ge(sem_pe, N_PREP_MM + 8 * (g - RBUF + 1))

        def first_waits(eng, need):
            # hj full + hb ready before any relu
            for sem_, n in need:
                eng.wait_ge(sem_, n)

        @block.sync
        def _(sp):
            sp.dma_start(xtb_sb[:, 0:512], xtb[:, 0:512]).then_inc(sem_xta, 16)
            sp.dma_start(xtb_sb[:, 512:N], xtb[:, 512:N]).then_inc(sem_xtb, 16)
            sp.dma_start(xtb_sb[:, N:XTB_W], xtb[:, N:XTB_W]).then_inc(sem_xt2, 16)
            sp.dma_start(madj_sb[:, 1024:2048], madj[:, 1024:2048]).then_inc(sem_ma1, 16)
            for nsig, ibase, w in OUT_PIECES[:-1]:
                sp.wait_ge(sem_sig, nsig)
                sp.dma_start(
                    out_v[:, :, ibase:ibase + w], o_v[:, :, ibase:ibase + w],
                ).then_inc(sem_out, 16)

        @block.gpsimd
        def _(gp):
            gp.memset(wmm_sb[:], 0.0).then_inc(sem_pms, 1)
            gp.wait_ge(sem_wb, 16)
            gp.dma_start(madj_sb[:, 0:1024], madj[:, 0:1024]).then_inc(sem_ma0, 16)
            nc.gpsimd.tensor_tensor(
                cvt_sb[:, 0:1], wb_sb[:, BV_O:BV_O + 1], wb_sb[:, BV_O + 1:BV_O + 2],
                ALU.add,
            )
            nc.gpsimd.tensor_copy(cvt_sb[:, 2:3], wb_sb[:, BV_O + 2:BV_O + 3]).then_inc(sem_cvt, 1)
            first = True
            for i_, g in enumerate(POOL_GG):
                if first:
                    # half-a needs only hj h0 + hb; half-b waits the h1 copy
                    first_waits(gp, [(sem_h0, 1), (sem_hb, 1)])
                    nc.gpsimd.tensor_scalar(
                        r_sb[g % RBUF][:, 0:512], hj_sb[:, 0:512],
                        hb_sb[:, g:g + 1], 0.0, ALU.add, ALU.max,
                    ).then_inc(sem_pR, 1)
                    gp.wait_ge(sem_vec, 1)
                    nc.gpsimd.tensor_scalar(
                        r_sb[g % RBUF][:, 512:1024], hj_sb[:, 512:1024],
                        hb_sb[:, g:g + 1], 0.0, ALU.add, ALU.max,
                    ).then_inc(sem_pR, 1)
                    first = False
                    continue
                r_war_wait(gp, g)
                nc.gpsimd.tensor_scalar(
                    r_sb[g % RBUF][:], hj_sb[:], hb_sb[:, g:g + 1], 0.0,
                    ALU.add, ALU.max,
                ).then_inc(sem_pR, 1)
                if i_ == 2:
                    gp.wait_ge(sem_ma1, 16)
                    nc.gpsimd.tensor_scalar(
                        m_sb[:, 1536:2048], madj_sb[:, 1536:2048], -1.0, 1e9,
                        ALU.add, ALU.mult,
                    ).then_inc(sem_mop2, 1)

        @block.tensor
        def _(pe):
            pe.wait_ge(sem_pms, 1)
            nc.tensor.matmul(acc[0][0:2, 0:2], wmm_sb[:, 0:2], wmm_sb[:, 2:4])
            pe.wait_ge(sem_xta, 16)
            pe.wait_ge(sem_wb, 16)
            nc.tensor.matmul(acc[0][:], wc_a, xt_a[:, 0:512]).then_inc(sem_pe, 1)
            pe.wait_ge(sem_xtb, 16)
            nc.tensor.matmul(acc[1][:], wc_a, xt_a[:, 512:1024]).then_inc(sem_pe, 1)
            pe.wait_ge(sem_xt2, 16)
            xt2_v = xtb_sb[:, N:XTB_W].rearrange("p (g k) -> p g k", k=4)
            for k in range(4):
                nc.tensor.matmul(
                    acc[2][32 * k:32 * (k + 1), 0:NG], wci_a, xt2_v[:, :, k],
                    tile_position=(0, 32 * k),
                ).then_inc(sem_pe, 1)
            # main: R stationary vs sign strip; bank cols NJ*js + 4*glh' + k.
            # Each sub-bank is seeded with the mask bias M (start=True) first.
            for rb, (g0, ngr, bw, mcb) in enumerate(BANKS):
                nj = bw // 8
                if rb == 0:
                    pe.wait_ge(sem_mop, 1)
                    pe.wait_ge(sem_h0, 1)   # copy h0 read of acc0 done
                elif rb == 1:
                    pe.wait_ge(sem_vec, 1)  # copy h1 read of acc1 done
                elif rb == 2:
                    pe.wait_ge(sem_mop, 2)
                    pe.wait_ge(sem_mop2, 1)
                    pe.wait_ge(sem_hb, 1)   # hb-add read of acc2 done
                nc.tensor.matmul(
                    acc[rb][:, 0:bw], id_a, m_sb[:, mcb:mcb + bw],
                    start=True, stop=False, skip_group_check=True,
                )
                for g in range(g0, g0 + ngr):
                    glh = g - g0
                    if _assign[g] == "a":
                        pe.wait_ge(sem_sR, scnt[g])
                    elif _assign[g] == "p":
                        pe.wait_ge(sem_pR, pcnt[g] - (1 if g == pool_first else 0))
                    else:
                        pe.wait_ge(sem_vR, vcnt[g])
                    r = r_sb[g % RBUF]
                    for js in range(8):
                        if g == pool_first and js == 4:
                            pe.wait_ge(sem_pR, pcnt[g])
                        col = nj * js + 4 * glh
                        last = glh == ngr - 1 and js == 7
                        nc.tensor.matmul(
                            acc[rb][:, col:col + 4],
                            r[:, 128 * js:128 * (js + 1)], sgn_a,
                            start=False, stop=last, skip_group_check=True,
                        ).then_inc(sem_pe, 1)

        @block.scalar
        def _(sc):
            sc.dma_start(wb_sb[:], wb[:]).then_inc(sem_wb, 16)
            nc.scalar.memzero(warm_sb[:]).then_inc(sem_warm, 1)
            sc.wait_ge(sem_warm, 1)
            nc.scalar.activation(warm_sb[:], warm_sb[:], AF.Sigmoid)
            sc.wait_ge(sem_pe, 1)
            nc.scalar.activation(
                hj_sb[:, 0:512], acc[0][:], AF.Copy,
            ).then_inc(sem_h0, 1)
            sc.wait_ge(sem_cvt, 1)
            sc.wait_ge(sem_pe, N_PREP_MM)
            nc.scalar.activation(
                hb_sb[:], acc[2][:, 0:NG], AF.Identity, bias=bvh_a, scale=1.0,
            ).then_inc(sem_hb, 1)

            sig_i = [0]

            def sigs(now_g):
                while sig_i[0] < len(SIG_CHUNKS):
                    g_end, rb, ib, w = SIG_CHUNKS[sig_i[0]]
                    lead = 5 if sig_i[0] < 4 else 2
                    if now_g < g_end + lead and now_g < NG:
                        return
                    if sig_i[0] == 0:
                        sc.wait_ge(sem_cvt, 1)
                    sc.wait_ge(sem_pe, N_PREP_MM + 8 * (g_end + 1))
                    bw = BANKS[rb][2]
                    nc.scalar.activation(
                        o_v[:, :, ib:ib + w], acc[rb][:, 0:bw],
                        AF.Sigmoid, bias=b2_a, scale=1.0,
                    ).then_inc(sem_sig, 1)
                    sig_i[0] += 1

            first = True
            for g in SCA_GG:
                if first:
                    first_waits(sc, [(sem_h0, 1), (sem_vec, 1), (sem_hb, 1)])
                    first = False
                r_war_wait(sc, g)
                nc.scalar.activation(
                    r_sb[g % RBUF][:], hj_sb[:], AF.Relu,
                    bias=hb_sb[:, g:g + 1], scale=1.0,
                ).then_inc(sem_sR, 1)
                sigs(g)
            sigs(NG)
            nsig, ibase, w = OUT_PIECES[-1]
            sc.wait_ge(sem_sig, nsig)
            sc.dma_start(
                out_v[:, :, ibase:ibase + w], o_v[:, :, ibase:ibase + w],
            ).then_inc(sem_out, 16)

        @block.vector
        def _(ve):
            ve.wait_ge(sem_pe, 2)
            nc.vector.tensor_copy(
                hj_sb[:, 512:1024], acc[1][:],
            ).then_inc(sem_vec, 1)
            mop_i = [0]

            def mops(now_g):
                # M = (adj - 1) * 1e9 in bf16, halves gated on the madj DMAs
                if mop_i[0] == 0 and now_g >= 5:
                    ve.wait_ge(sem_ma0, 16)
                    nc.vector.tensor_scalar(
                        m_sb[:, 0:1024], madj_sb[:, 0:1024], -1.0, 1e9,
                        ALU.add, ALU.mult,
                    ).then_inc(sem_mop, 1)
                    mop_i[0] = 1
                if mop_i[0] == 1 and now_g >= 20:
                    ve.wait_ge(sem_ma1, 16)
                    nc.vector.tensor_scalar(
                        m_sb[:, 1024:1536], madj_sb[:, 1024:1536], -1.0, 1e9,
                        ALU.add, ALU.mult,
                    ).then_inc(sem_mop, 1)
                    mop_i[0] = 2

            first = True
            for g in VEC_GG:
                if first:
                    first_waits(ve, [(sem_h0, 1), (sem_vec, 1), (sem_hb, 1)])
                    first = False
                r_war_wait(ve, g)
                nc.vector.tensor_scalar(
                    r_sb[g % RBUF][:], hj_sb[:], hb_sb[:, g:g + 1], 0.0,
                    ALU.add, ALU.max,
                ).then_inc(sem_vR, 1)
                mops(g)
            mops(NG)

    return nc


def _host_prep(node_features, adjacency_matrix, Wt, bt, W1, b1, W2, b2):
    """Build per-core input maps (numpy only: resharding + weight algebra)."""
    import ml_dtypes

    f = np.float32
    bf = ml_dtypes.bfloat16
    W2v = np.asarray(W2, f)[0]                 # [E]
    aW2 = np.abs(W2v)
    sW2 = np.sign(W2v).astype(f)
    W1 = np.asarray(W1, f)
    W1i, W1j = W1[:, :H], W1[:, H:]            # [E, H]
    Wt = np.asarray(Wt, f)                     # [o, h]
    bt = np.asarray(bt, f)
    b1 = np.asarray(b1, f)
    b2c = float(np.asarray(b2, f).reshape(-1)[0])

    WtW1j = W1j @ Wt                           # [E, H]
    WtW1i = W1i @ Wt
    allb = b1 + W1j @ bt + W1i @ bt            # [E] constant part of relu arg

    bvec = np.tile(aW2 * allb, 4).astype(f)    # [128]
    bv_hi = bvec.astype(bf)
    bv_lo = (bvec - bv_hi.astype(f)).astype(bf)

    wb = np.zeros((128, WB_W), bf)
    for k in range(4):
        wb[32 * k:32 * (k + 1), SGN_O + k] = sW2.astype(bf)
    wb[:, WC_O:WC_O + 128] = np.tile((aW2[:, None] * WtW1j).T, (1, 4)).astype(bf)
    wb[:, WCI_O:WCI_O + E] = (aW2[:, None] * WtW1i).T.astype(bf)
    wb[:, BV_O] = bv_hi
    wb[:, BV_O + 1] = bv_lo
    wb[:, BV_O + 2] = bf(b2c)
    wb[:, ID_O:ID_O + 128] = np.eye(128, dtype=bf)

    in_maps = []
    adjacency_matrix = np.asarray(adjacency_matrix, f)
    node_features = np.asarray(node_features, f)
    for c in range(NCORES):
        b, i0 = c // (NCORES // B), ROWS_PER_CORE * (c % (NCORES // B))
        x = node_features[b]
        xtb = np.empty((128, XTB_W), bf)
        xtb[:, 0:N] = x.T.astype(bf)
        xtb[:, N:] = x[i0:i0 + ROWS_PER_CORE].T.astype(bf)
        # madj[j_loc, mcb + NJ*js + 4*glh' + k] = adj[i0 + i_loc, 128*js + j_loc]
        # with i_loc = 4*(g0 + glh') + k for each sub-bank (g0, ngr, bw, mcb)
        adjT = adjacency_matrix[i0:i0 + ROWS_PER_CORE].T.astype(bf)  # [j, i_loc]
        A = adjT.reshape(8, 128, 64, 4)  # js, j_loc, g, k
        madj = np.empty((128, 2048), bf)
        for g0, ngr, bw, mcb in BANKS:
            blk = A[:, :, g0:g0 + ngr, :]          # js, j_loc, glh', k
            blk = blk.transpose(1, 0, 2, 3).reshape(128, bw)  # j_loc, (js, glh', k)
            madj[:, mcb:mcb + bw] = blk
        in_maps.append(dict(xtb=xtb, wb=wb, madj=madj))
    return in_maps


def run(inputs, trace=False):
    from concourse.bass_utils import run_bass_kernel_spmd

    if "prog" not in _CACHE:
        _CACHE["prog"] = _build_program()
    nc = _CACHE["prog"]
    in_maps = _host_prep(**inputs)
    res = run_bass_kernel_spmd(nc, in_maps, list(range(NCORES)), trace=trace)
    out = np.empty((B, N, N), np.float32)
    for c in range(NCORES):
        b, i0 = c // (NCORES // B), ROWS_PER_CORE * (c % (NCORES // B))
        out[b, i0:i0 + ROWS_PER_CORE] = res.results[c]["out"].T
    return out, res


def kernel(**inputs):
    out, _ = run(inputs, trace=False)
    return out


# revision 27
# speedup vs baseline: 1.0078x; 1.0054x over previous
"""Trainium2 Bass kernel for nn_MetaGraphLearner (GNN edge scorer).

Math (reference):
  t  = X @ Wt.T + bt                  [B,N,H]
  hi = t @ W1i.T, hj = t @ W1j.T      [B,N,E]   (W1 = [W1i | W1j])
  ew[b,i,j] = sum_e W2[e]*relu(hi[b,i,e]+hj[b,j,e]+b1[e]) + b2
  out = sigmoid(ew) * adj[None]

Wt is folded into W1 host-side (weight algebra only):
  hj[j,e] = X[j] @ (W1j@Wt)[e], hi likewise; all constant terms fold into one
  per-(k,e) bias vector bvec (shipped as a bf16 hi/lo pair, recombined in f32).

Kernel (per core, 8 cores, each owns 256 of the B*N=2048 rows):
  Partition layout p = 32*k + e  (k in 0..3 row-sublane, e in 0..31).
  hjT4[p, j]  = |W2[e]|*hj_lin[j,e]             [128,1024] bf16 (2 matmuls +
                PSUM->SBUF copies on Act/DVE)
  hb[p, g]    = |W2[e]|*hi_lin[4g+k,e] + bvec   [128,64] f32 (4 matmuls + an
                Identity activation w/ bias on Act)
  R_g[p, j]   = max(hjT4[p,j] + hb[p,g], 0)     64 groups, split 39/10/15
                across DVE (4x bf16 mode, 327ns), Act (relu w/ bias, 1038ns)
                and GpSimd/Pool (853ns) to finish simultaneously
  psum[i', j] += sign(W2[e]) * R_g[p,j]         8 bf16 matmuls per group into
                six PSUM sub-banks (BANKS table) sized 16/16/16/8/4/4 groups
                so output chunks complete at g15/31/47/55/59/63
  mask        : since adj is 0/1, sigmoid(ew)*adj == sigmoid(ew - 1e9*(1-adj));
                M = (adj-1)*1e9 is computed from bf16 adj (DVE 4x, one slice
                on Pool) and seeded into each PSUM sub-bank by ONE
                identity-stationary matmul (start=True) BEFORE the group
                matmuls accumulate (start=False). No mask multiply exists.
  out         = sigmoid(psum + b2)  ->  o_sb [j_loc, 256*js + i'], six
                activation chunks firing as sub-banks complete; 5 output DMAs
                cascade so only a tiny [*, 16-col] piece (issued by Act right
                after the last sigmoid) remains after g63.

Cost-model notes (CoreSim): matmuls are charged out-free-size * 0.42ns only
(stationary load unmodeled) so PE is nearly free; DMA HWDGE descriptor-gen
(625ns) and the transfer pool are globally serialized, hence few, ordered
DMAs with madj routed via Pool SWDGE; every DMA completion costs +900ns sem
propagation. Walrus constraints found by probing: GpSimd must not touch PSUM;
activation inputs must collapse to <=2D APs.

Raw Bass (explicit engine blocks + semaphores): this walrus build rejects >1
attached sync wait per instruction, so cross-engine waits are standalone
wait_ge instructions.
"""

import sys

if "/opt/trn_rl_repo" not in sys.path:
    sys.path.insert(0, "/opt/trn_rl_repo")

import numpy as np
from contextlib import ExitStack

B, N, H, E = 2, 1024, 128, 32
NCORES = 8
ROWS_PER_CORE = (B * N) // NCORES  # 256
NG = 64                   # groups of 4 rows per core
RBUF = 12                 # R-tile ring slots
NSCA = 10                 # relu tiles on ScalarE (Act)
NPOOL = 15                # relu tiles on GpSimd (Pool)

# bf16 input "xtb" free-axis layout: [ X.T (N) | X2.T (256) ]
XTB_W = N + ROWS_PER_CORE
# bf16 input "wb" layout: sgn(4) | wc(128) | wci(32) | bv_hi | bv_lo | b2 | ident(128)
SGN_O, WC_O, WCI_O, BV_O, ID_O = 0, 4, 132, 164, 167
WB_W = ID_O + 128  # 295

N_PREP_MM = 6  # 2 hj + 4 hb (these inc sem_pe; M-matmuls do not)

# --- relu group -> engine assignment -----------------------------------------
# deterministic largest-deficit interleave with exact counts
_counts = {"v": NG - NSCA - NPOOL, "a": NSCA, "p": NPOOL}
_assign = []
_used = {"v": 0, "a": 0, "p": 0}
for g in range(NG):
    best = max(("v", "a", "p"), key=lambda e: (_counts[e] * (g + 1) / NG - _used[e], e))
    _assign.append(best)
    _used[best] += 1
# force the last 3 groups onto DVE (fastest) so the tail starts ASAP
for g in (60, 61, 62, 63):
    if _assign[g] != "v":
        for g2 in range(60, -1, -1):
            if _assign[g2] == "v":
                _assign[g2], _assign[g] = _assign[g], "v"
                break
SCA_GG = [g for g in range(NG) if _assign[g] == "a"]
POOL_GG = [g for g in range(NG) if _assign[g] == "p"]
VEC_GG = [g for g in range(NG) if _assign[g] == "v"]

# PSUM sub-banks (each its own physical 2KB bank so the M-seed + walrus's
# 2D-activation-input constraint hold): (first group, #groups, bank cols, madj col base)
# within-bank col = (#groups/2)*js*?? -> col = NJ*js + 4*glh' + k with NJ = 4*ngroups/8*4
BANKS = [
    (0, 16, 512, 0),       # slab0 gh0: groups 0-15
    (16, 16, 512, 512),    # slab0 gh1
    (32, 16, 512, 1024),   # slab1 gh0
    (48, 8, 256, 1536),    # slab1 gh1 glh 0-7
    (56, 4, 128, 1792),    # glh 8-11
    (60, 4, 128, 1920),    # glh 12-15
]
# sigmoid chunks, one per bank: (g_end, bank idx, o_sb i-col base, i width)
SIG_CHUNKS = [
    (15, 0, 0, 64),
    (31, 1, 64, 64),
    (47, 2, 128, 64),
    (55, 3, 192, 32),
    (59, 4, 224, 16),
    (63, 5, 240, 16),
]
# out DMA pieces: (#sig chunks required, o_sb i-col base, width)
OUT_PIECES = [(2, 0, 128), (3, 128, 64), (4, 192, 32), (5, 224, 16), (6, 240, 16)]

_CACHE = {}


def _build_program():
    import concourse.bass as bass
    import concourse.mybir as mybir

    f32 = mybir.dt.float32
    bf16 = mybir.dt.bfloat16
    AF = mybir.ActivationFunctionType
    ALU = mybir.AluOpType

    nc = bass.Bass()
    xtb = nc.declare_dram_parameter("xtb", [128, XTB_W], bf16, isOutput=False)
    wb = nc.declare_dram_parameter("wb", [128, WB_W], bf16, isOutput=False)
    madj = nc.declare_dram_parameter("madj", [128, 2048], bf16, isOutput=False)
    out_d = nc.declare_dram_parameter("out", [N, ROWS_PER_CORE], f32, isOutput=True)

    # per-engine producer ordinals for the relu ring. Pool's first group is
    # produced as two halves (two sem_pR increments).
    vcnt, scnt, pcnt = {}, {}, {}
    v = s = p = 0
    pool_first = POOL_GG[0]
    for g in range(NG):
        if _assign[g] == "a":
            s += 1
            scnt[g] = s
        elif _assign[g] == "p":
            p += 2 if g == pool_first else 1
            pcnt[g] = p
        else:
            v += 1
            vcnt[g] = v

    with ExitStack() as ctx:
        EN = ctx.enter_context
        xtb_sb = EN(nc.sbuf_tensor("xtb_sb", [128, XTB_W], bf16))
        wb_sb = EN(nc.sbuf_tensor("wb_sb", [128, WB_W], bf16))
        cvt_sb = EN(nc.sbuf_tensor("cvt_sb", [128, 3], f32))
        warm_sb = EN(nc.sbuf_tensor("warm_sb", [128, 1], f32))
        wmm_sb = EN(nc.sbuf_tensor("wmm_sb", [128, 4], bf16))
        hj_sb = EN(nc.sbuf_tensor("hj_sb", [128, N], bf16))
        hb_sb = EN(nc.sbuf_tensor("hb_sb", [128, NG], f32))
        madj_sb = EN(nc.sbuf_tensor("madj_sb", [128, 2048], bf16))
        m_sb = EN(nc.sbuf_tensor("m_sb", [128, 2048], bf16))
        r_sb = [EN(nc.sbuf_tensor(f"r{i}", [128, N], bf16)) for i in range(RBUF)]
        o_sb = EN(nc.sbuf_tensor("o_sb", [128, 2048], f32))

        acc = [EN(nc.psum_tensor(f"acc{i}", [128, 512], f32)) for i in range(6)]

        sem_xta = EN(nc.semaphore("sxta"))
        sem_xtb = EN(nc.semaphore("sxtb"))
        sem_xt2 = EN(nc.semaphore("sxt2"))
        sem_wb = EN(nc.semaphore("swb"))
        sem_ma0 = EN(nc.semaphore("sma0"))
        sem_ma1 = EN(nc.semaphore("sma1"))
        sem_cvt = EN(nc.semaphore("scvt"))
        sem_hb = EN(nc.semaphore("shb"))
        sem_mop = EN(nc.semaphore("smop"))
        sem_mop2 = EN(nc.semaphore("smop2"))
        sem_pe = EN(nc.semaphore("spe"))
        sem_h0 = EN(nc.semaphore("sh0"))
        sem_h1a = EN(nc.semaphore("sh1a"))
        sem_vec = EN(nc.semaphore("svec"))
        sem_sig = EN(nc.semaphore("ssig"))
        sem_vR = EN(nc.semaphore("svr"))
        sem_sR = EN(nc.semaphore("ssr"))
        sem_pR = EN(nc.semaphore("spr"))
        sem_warm = EN(nc.semaphore("swarm"))
        sem_pms = EN(nc.semaphore("spms"))
        sem_out = EN(nc.semaphore("sout"))

        xt_a = xtb_sb[:, 0:N]
        sgn_a = wb_sb[:, SGN_O:SGN_O + 4]
        wc_a = wb_sb[:, WC_O:WC_O + 128]
        wci_a = wb_sb[:, WCI_O:WCI_O + E]
        id_a = wb_sb[:, ID_O:ID_O + 128]
        bvh_a = cvt_sb[:, 0:1]
        bvl_a = cvt_sb[:, 1:2]
        b2_a = cvt_sb[:, 2:3]

        o_v = o_sb[:].rearrange("p (js i) -> p js i", js=8)
        out_v = out_d[:].rearrange("(js p) i -> p js i", js=8)

        block = EN(nc.Block())

        def r_war_wait(eng, g):
            # overwrite slot g%RBUF: previous tenant g-RBUF must be consumed
            if g >= RBUF:
                eng.wait_ge(sem_pe, N_PREP_MM + 8 * (g - RBUF + 1))

        def first_waits(eng, need):
            # hj full + hb ready before any relu
            for sem_, n in need:
                eng.wait_ge(sem_, n)

        @block.sync
        def _(sp):
            sp.dma_start(xtb_sb[:, 0:512], xtb[:, 0:512]).then_inc(sem_xta, 16)
            sp.dma_start(xtb_sb[:, 512:N], xtb[:, 512:N]).then_inc(sem_xtb, 16)
            sp.dma_start(xtb_sb[:, N:XTB_W], xtb[:, N:XTB_W]).then_inc(sem_xt2, 16)
            sp.dma_start(madj_sb[:, 1024:2048], madj[:, 1024:2048]).then_inc(sem_ma1, 16)
            for nsig, ibase, w in OUT_PIECES[:-1]:
                sp.wait_ge(sem_sig, nsig)
                sp.dma_start(
                    out_v[:, :, ibase:ibase + w], o_v[:, :, ibase:ibase + w],
                ).then_inc(sem_out, 16)

        @block.gpsimd
        def _(gp):
            gp.memset(wmm_sb[:], 0.0).then_inc(sem_pms, 1)
            gp.wait_ge(sem_wb, 16)
            gp.dma_start(madj_sb[:, 0:1024], madj[:, 0:1024]).then_inc(sem_ma0, 16)
            nc.gpsimd.tensor_tensor(
                cvt_sb[:, 0:1], wb_sb[:, BV_O:BV_O + 1], wb_sb[:, BV_O + 1:BV_O + 2],
                ALU.add,
            )
            nc.gpsimd.tensor_copy(cvt_sb[:, 2:3], wb_sb[:, BV_O + 2:BV_O + 3]).then_inc(sem_cvt, 1)
            first = True
            for i_, g in enumerate(POOL_GG):
                if first:
                    # half-a needs only hj h0 + hb; half-b waits the h1 copy
                    first_waits(gp, [(sem_h0, 1), (sem_hb, 1)])
                    nc.gpsimd.tensor_scalar(
                        r_sb[g % RBUF][:, 0:512], hj_sb[:, 0:512],
                        hb_sb[:, g:g + 1], 0.0, ALU.add, ALU.max,
                    ).then_inc(sem_pR, 1)
                    gp.wait_ge(sem_vec, 2)
                    nc.gpsimd.tensor_scalar(
                        r_sb[g % RBUF][:, 512:1024], hj_sb[:, 512:1024],
                        hb_sb[:, g:g + 1], 0.0, ALU.add, ALU.max,
                    ).then_inc(sem_pR, 1)
                    first = False
                    continue
                r_war_wait(gp, g)
                nc.gpsimd.tensor_scalar(
                    r_sb[g % RBUF][:], hj_sb[:], hb_sb[:, g:g + 1], 0.0,
                    ALU.add, ALU.max,
                ).then_inc(sem_pR, 1)
                if i_ == 2:
                    gp.wait_ge(sem_ma1, 16)
                    nc.gpsimd.tensor_scalar(
                        m_sb[:, 1536:2048], madj_sb[:, 1536:2048], -1.0, 1e9,
                        ALU.add, ALU.mult,
                    ).then_inc(sem_mop2, 1)

        @block.tensor
        def _(pe):
            pe.wait_ge(sem_pms, 1)
            nc.tensor.matmul(acc[0][0:2, 0:2], wmm_sb[:, 0:2], wmm_sb[:, 2:4])
            pe.wait_ge(sem_xta, 16)
            pe.wait_ge(sem_wb, 16)
            nc.tensor.matmul(acc[0][:], wc_a, xt_a[:, 0:512]).then_inc(sem_pe, 1)
            pe.wait_ge(sem_xtb, 16)
            nc.tensor.matmul(acc[1][:, 0:256], wc_a, xt_a[:, 512:768]).then_inc(sem_h1a, 1)
            nc.tensor.matmul(acc[1][:, 256:512], wc_a, xt_a[:, 768:1024]).then_inc(sem_pe, 1)
            pe.wait_ge(sem_xt2, 16)
            xt2_v = xtb_sb[:, N:XTB_W].rearrange("p (g k) -> p g k", k=4)
            for k in range(4):
                nc.tensor.matmul(
                    acc[2][32 * k:32 * (k + 1), 0:NG], wci_a, xt2_v[:, :, k],
                    tile_position=(0, 32 * k),
                ).then_inc(sem_pe, 1)
            # main: R stationary vs sign strip; bank cols NJ*js + 4*glh' + k.
            # Each sub-bank is seeded with the mask bias M (start=True) first.
            for rb, (g0, ngr, bw, mcb) in enumerate(BANKS):
                nj = bw // 8
                if rb == 0:
                    pe.wait_ge(sem_mop, 1)
                    pe.wait_ge(sem_h0, 1)   # copy h0 read of acc0 done
                elif rb == 1:
                    pe.wait_ge(sem_vec, 2)  # both h1 copy reads of acc1 done
                elif rb == 2:
                    pe.wait_ge(sem_mop, 2)
                    pe.wait_ge(sem_mop2, 1)
                    pe.wait_ge(sem_hb, 1)   # hb-add read of acc2 done
                nc.tensor.matmul(
                    acc[rb][:, 0:bw], id_a, m_sb[:, mcb:mcb + bw],
                    start=True, stop=False, skip_group_check=True,
                )
                for g in range(g0, g0 + ngr):
                    glh = g - g0
                    if _assign[g] == "a":
                        pe.wait_ge(sem_sR, scnt[g])
                    elif _assign[g] == "p":
                        pe.wait_ge(sem_pR, pcnt[g] - (1 if g == pool_first else 0))
                    else:
                        pe.wait_ge(sem_vR, vcnt[g])
                    r = r_sb[g % RBUF]
                    for js in range(8):
                        if g == pool_first and js == 4:
                            pe.wait_ge(sem_pR, pcnt[g])
                        col = nj * js + 4 * glh
                        last = glh == ngr - 1 and js == 7
                        nc.tensor.matmul(
                            acc[rb][:, col:col + 4],
                            r[:, 128 * js:128 * (js + 1)], sgn_a,
                            start=False, stop=last, skip_group_check=True,
                        ).then_inc(sem_pe, 1)

        @block.scalar
        def _(sc):
            sc.dma_start(wb_sb[:], wb[:]).then_inc(sem_wb, 16)
            nc.scalar.memzero(warm_sb[:]).then_inc(sem_warm, 1)
            sc.wait_ge(sem_warm, 1)
            nc.scalar.activation(warm_sb[:], warm_sb[:], AF.Sigmoid)
            sc.wait_ge(sem_pe, 1)
            nc.scalar.activation(
                hj_sb[:, 0:512], acc[0][:], AF.Copy,
            ).then_inc(sem_h0, 1)
            sc.wait_ge(sem_cvt, 1)
            sc.wait_ge(sem_pe, N_PREP_MM)
            nc.scalar.activation(
                hb_sb[:], acc[2][:, 0:NG], AF.Identity, bias=bvh_a, scale=1.0,
            ).then_inc(sem_hb, 1)

            sig_i = [0]

            def sigs(now_g):
                while sig_i[0] < len(SIG_CHUNKS):
                    g_end, rb, ib, w = SIG_CHUNKS[sig_i[0]]
                    lead = 5 if sig_i[0] < 4 else 2
                    if now_g < g_end + lead and now_g < NG:
                        return
                    if sig_i[0] == 0:
                        sc.wait_ge(sem_cvt, 1)
                    sc.wait_ge(sem_pe, N_PREP_MM + 8 * (g_end + 1))
                    bw = BANKS[rb][2]
                    nc.scalar.activation(
                        o_v[:, :, ib:ib + w], acc[rb][:, 0:bw],
                        AF.Sigmoid, bias=b2_a, scale=1.0,
                    ).then_inc(sem_sig, 1)
                    sig_i[0] += 1

            first = True
            for g in SCA_GG:
                if first:
                    first_waits(sc, [(sem_h0, 1), (sem_vec, 2), (sem_hb, 1)])
                    first = False
                r_war_wait(sc, g)
                nc.scalar.activation(
                    r_sb[g % RBUF][:], hj_sb[:], AF.Relu,
                    bias=hb_sb[:, g:g + 1], scale=1.0,
                ).then_inc(sem_sR, 1)
                sigs(g)
            sigs(NG)
            nsig, ibase, w = OUT_PIECES[-1]
            sc.wait_ge(sem_sig, nsig)
            sc.dma_start(
                out_v[:, :, ibase:ibase + w], o_v[:, :, ibase:ibase + w],
            ).then_inc(sem_out, 16)

        @block.vector
        def _(ve):
            ve.wait_ge(sem_h1a, 1)
            nc.vector.tensor_copy(
                hj_sb[:, 512:768], acc[1][:, 0:256],
            ).then_inc(sem_vec, 1)
            ve.wait_ge(sem_pe, 2)
            nc.vector.tensor_copy(
                hj_sb[:, 768:1024], acc[1][:, 256:512],
            ).then_inc(sem_vec, 1)
            mop_i = [0]

            def mops(now_g):
                # M = (adj - 1) * 1e9 in bf16, halves gated on the madj DMAs
                if mop_i[0] == 0 and now_g >= 5:
                    ve.wait_ge(sem_ma0, 16)
                    nc.vector.tensor_scalar(
                        m_sb[:, 0:1024], madj_sb[:, 0:1024], -1.0, 1e9,
                        ALU.add, ALU.mult,
                    ).then_inc(sem_mop, 1)
                    mop_i[0] = 1
                if mop_i[0] == 1 and now_g >= 20:
                    ve.wait_ge(sem_ma1, 16)
                    nc.vector.tensor_scalar(
                        m_sb[:, 1024:1536], madj_sb[:, 1024:1536], -1.0, 1e9,
                        ALU.add, ALU.mult,
                    ).then_inc(sem_mop, 1)
                    mop_i[0] = 2

            first = True
            for g in VEC_GG:
                if first:
                    first_waits(ve, [(sem_h0, 1), (sem_vec, 2), (sem_hb, 1)])
                    first = False
                r_war_wait(ve, g)
                nc.vector.tensor_scalar(
                    r_sb[g % RBUF][:], hj_sb[:], hb_sb[:, g:g + 1], 0.0,
                    ALU.add, ALU.max,
                ).then_inc(sem_vR, 1)
                mops(g)
            mops(NG)

    return nc


def _host_prep(node_features, adjacency_matrix, Wt, bt, W1, b1, W2, b2):
    """Build per-core input maps (numpy only: resharding + weight algebra)."""
    import ml_dtypes

    f = np.float32
    bf = ml_dtypes.bfloat16
    W2v = np.asarray(W2, f)[0]                 # [E]
    aW2 = np.abs(W2v)
    sW2 = np.sign(W2v).astype(f)
    W1 = np.asarray(W1, f)
    W1i, W1j = W1[:, :H], W1[:, H:]            # [E, H]
    Wt = np.asarray(Wt, f)                     # [o, h]
    bt = np.asarray(bt, f)
    b1 = np.asarray(b1, f)
    b2c = float(np.asarray(b2, f).reshape(-1)[0])

    WtW1j = W1j @ Wt                           # [E, H]
    WtW1i = W1i @ Wt
    allb = b1 + W1j @ bt + W1i @ bt            # [E] constant part of relu arg

    bvec = np.tile(aW2 * allb, 4).astype(f)    # [128]
    bv_hi = bvec.astype(bf)
    bv_lo = (bvec - bv_hi.astype(f)).astype(bf)

    wb = np.zeros((128, WB_W), bf)
    for k in range(4):
        wb[32 * k:32 * (k + 1), SGN_O + k] = sW2.astype(bf)
    wb[:, WC_O:WC_O + 128] = np.tile((aW2[:, None] * WtW1j).T, (1, 4)).astype(bf)
    wb[:, WCI_O:WCI_O + E] = (aW2[:, None] * WtW1i).T.astype(bf)
    wb[:, BV_O] = bv_hi
    wb[:, BV_O + 1] = bv_lo
    wb[:, BV_O + 2] = bf(b2c)
    wb[:, ID_O:ID_O + 128] = np.eye(128, dtype=bf)

    in_maps = []
    adjacency_matrix = np.asarray(adjacency_matrix, f)
    node_features = np.asarray(node_features, f)
    for c in range(NCORES):
        b, i0 = c // (NCORES // B), ROWS_PER_CORE * (c % (NCORES // B))
        x = node_features[b]
        xtb = np.empty((128, XTB_W), bf)
        xtb[:, 0:N] = x.T.astype(bf)
        xtb[:, N:] = x[i0:i0 + ROWS_PER_CORE].T.astype(bf)
        # madj[j_loc, mcb + NJ*js + 4*glh' + k] = adj[i0 + i_loc, 128*js + j_loc]
        # with i_loc = 4*(g0 + glh') + k for each sub-bank (g0, ngr, bw, mcb)
        adjT = adjacency_matrix[i0:i0 + ROWS_PER_CORE].T.astype(bf)  # [j, i_loc]
        A = adjT.reshape(8, 128, 64, 4)  # js, j_loc, g, k
        madj = np.empty((128, 2048), bf)
        for g0, ngr, bw, mcb in BANKS:
            blk = A[:, :, g0:g0 + ngr, :]          # js, j_loc, glh', k
            blk = blk.transpose(1, 0, 2, 3).reshape(128, bw)  # j_loc, (js, glh', k)
            madj[:, mcb:mcb + bw] = blk
        in_maps.append(dict(xtb=xtb, wb=wb, madj=madj))
    return in_maps


def run(inputs, trace=False):
    from concourse.bass_utils import run_bass_kernel_spmd

    if "prog" not in _CACHE:
        _CACHE["prog"] = _build_program()
    nc = _CACHE["prog"]
    in_maps = _host_prep(**inputs)
    res = run_bass_kernel_spmd(nc, in_maps, list(range(NCORES)), trace=trace)
    out = np.empty((B, N, N), np.float32)
    for c in range(NCORES):
        b, i0 = c // (NCORES // B), ROWS_PER_CORE * (c % (NCORES // B))
        out[b, i0:i0 + ROWS_PER_CORE] = res.results[c]["out"].T
    return out, res


def kernel(**inputs):
    out, _ = run(inputs, trace=False)
    return out
